# revision 1
# baseline (speedup 1.0000x reference)
"""Cross-attention kernel for 8 trn2 NeuronCores.

Reference computation (per batch b of 16):
  q = Wq @ x, k = Wk @ y, v = Wv @ y          (1x1 convs as channel matmuls)
  q,k l2-normalized over the SPATIAL axis (per (h,d) row)
  sim = 10 * q^T k per head; attn = softmax_j(sim); o = attn @ v^T
  out = Wo @ o + b

Sharding: data-parallel over batch, 2 batches per core, weights replicated.

v3 design (ACT-bound; optimized against the TimelineSim cost model):
  - exp on ACT is the roofline: 64 x [128,1024] tiles = 66.4us busy/core.
  - S_T (q^T k, d=64 contraction) in fp8e4m3 + MatmulPerfMode.DoubleRow
    (0.5 cycles/row, dst partition base must be 0): q quantized RAW (q~N(0,1)
    fits e4m3), k carries the combined l2 scale sq*sk*2^10 (power-of-two gain
    keeps k8 in range; exp scale becomes 10/1024). DoubleRow pair layout
    [32p, 2pair, n] per head (d = p + 32*pair) via a DRAM-scratch reshuffle.
  - softmax denominator: 64-wide ones block in v^T puts the denominator
    pre-broadcast on PV out rows 64:128; normalize = reciprocal + multiply
    per i-half (DVE, PSUM-sourced).
  - zproj contracts kc1 as two 64-row matmuls (heads 2 and 3 separately) so
    the drain tail only waits on the last head's 64-row matmul.
  - Cross-head S_T handoff: head h emits head h+1's first S_T before its own
    last PV, so ACT never gaps at head boundaries (st ring bufs=2 fits this).
  - PSUM (16KB/partition): st [128,1024]f32 x2 (8K) + acc [128,512]f32 x2
    (4K) + op [128,1024]f32 x1 (4K).
  - Weights packed into 2 DMAs (wqk, wvo); batch-1 x/y loads deferred into
    head (0,0) so the fp8 reshuffle round-trip owns the DMA engine early.
"""

import sys

import numpy as np

if "/opt/trn_rl_repo" not in sys.path:
    sys.path.insert(0, "/opt/trn_rl_repo")

NB = 2        # batches per core
C = 256       # channels
N = 1024      # spatial (32*32)
HEADS = 4
DH = 64
HID = 256
NCORES = 8
MAGIC = 0x5F3759DF  # Quake fast inverse-sqrt seed
KGAIN = 1024.0      # power-of-two gain folded into k8; exp scale = 10/KGAIN

_CACHE = {}
PHASES = []


def _mark(nc, label):
    PHASES.append((int(nc.get_next_instruction_name()[2:]), label))


def _quake_rsqrt(eng, pool, p_ap, out_ap, final_scale):
    """out = rsqrt(p) * final_scale for [128,1] fp32 APs on engine `eng`.

    Quake seed + 2 Newton iterations (rel err ~1e-7), no ACT table needed.
    """
    from concourse import mybir

    i32 = mybir.dt.int32
    alu = mybir.AluOpType
    t = pool.tile([128, 1], mybir.dt.float32, tag="qk_rs_t", bufs=4)
    r = pool.tile([128, 1], mybir.dt.float32, tag="qk_rs_r", bufs=4)
    a = pool.tile([128, 1], mybir.dt.float32, tag="qk_rs_a", bufs=4)
    # seed: r0 = bitcast(MAGIC - (bitcast_i32(p) >> 1))
    eng.tensor_scalar(t.bitcast(i32), p_ap.bitcast(i32), 1, None,
                      alu.logical_shift_right)
    eng.tensor_scalar(r.bitcast(i32), t.bitcast(i32), -1, MAGIC,
                      alu.mult, alu.add)
    # Newton 1: r = r * (1.5 - 0.5 * p * r^2)
    eng.scalar_tensor_tensor(a[:], r[:], r[:, 0:1], p_ap,
                             alu.mult, alu.mult)
    eng.tensor_scalar(a[:], a[:], -0.5, 1.5, alu.mult, alu.add)
    eng.tensor_scalar(t[:], a[:], r[:, 0:1], None, alu.mult)
    # Newton 2 (fold final_scale into the last multiply)
    eng.scalar_tensor_tensor(a[:], t[:], t[:, 0:1], p_ap,
                             alu.mult, alu.mult)
    eng.tensor_scalar(a[:], a[:], -0.5, 1.5, alu.mult, alu.add)
    eng.tensor_scalar(out_ap, a[:], t[:, 0:1], final_scale,
                      alu.mult, alu.mult)


def _build_nc():
    from contextlib import ExitStack

    import concourse.tile as tile
    from concourse import bacc, mybir

    f32 = mybir.dt.float32
    f16 = mybir.dt.float16
    f8 = mybir.dt.float8e4
    alu = mybir.AluOpType
    EXP = mybir.ActivationFunctionType.Exp
    DR = mybir.MatmulPerfMode.DoubleRow

    nc = bacc.Bacc("TRN2", target_bir_lowering=False)

    xin = nc.dram_tensor("x", [NB, C, N], f16, kind="ExternalInput")
    yin = nc.dram_tensor("y", [NB, C, N], f16, kind="ExternalInput")
    wqk = nc.dram_tensor("wqk", [128, 4, HID], f16, kind="ExternalInput")
    wvo = nc.dram_tensor("wvo", [128, 4, HID], f16, kind="ExternalInput")
    bo = nc.dram_tensor("b_out", [2, 128, 1], f32, kind="ExternalInput")
    out = nc.dram_tensor("out", [NB, C, N], f32, kind="ExternalOutput")
    # DRAM scratch for the fp8 DoubleRow pair-layout reshuffle
    q8d = nc.dram_tensor("q8_scratch", [NB, 2, 64, 2 * N], f8, kind="Internal")
    k8d = nc.dram_tensor("k8_scratch", [NB, 2, 64, 2 * N], f8, kind="Internal")

    with tile.TileContext(nc) as tc, ExitStack() as ctx:
        consts = ctx.enter_context(tc.tile_pool(name="consts", bufs=1))
        big = ctx.enter_context(tc.tile_pool(name="big", bufs=2))
        sm = ctx.enter_context(tc.tile_pool(name="sm", bufs=4))
        ps = ctx.enter_context(tc.tile_pool(name="ps", bufs=2, space="PSUM"))

        # ---- weight + input loads ------------------------------------
        wqk_sb = consts.tile([128, 4, HID], f16, tag="wqk")
        wvo_sb = consts.tile([128, 4, HID], f16, tag="wvo")
        b_sb = consts.tile([128, 2, 1], f32, tag="bo")
        # warm the ACT exp table while input DMAs are in flight
        warm = sm.tile([128, 1], f32, tag="warm", bufs=1)
        nc.vector.memset(warm[:], 0.0)
        nc.scalar.activation(out=warm[:], in_=warm[:], func=EXP, scale=1.0)
        xts, yts = [], []
        for nb in range(NB):
            xt = big.tile([128, 2, N], f16, tag="xt", bufs=2)
            yt = big.tile([128, 2, N], f16, tag="yt", bufs=2)
            xts.append(xt)
            yts.append(yt)
        nc.sync.dma_start(out=wqk_sb[:], in_=wqk[:])
        nc.sync.dma_start(out=xts[0][:], in_=xin[0].rearrange("(kc p) n -> p kc n", p=128))
        nc.sync.dma_start(out=yts[0][:], in_=yin[0].rearrange("(kc p) n -> p kc n", p=128))
        nc.sync.dma_start(out=wvo_sb[:], in_=wvo[:])
        nc.sync.dma_start(out=b_sb[:], in_=bo.rearrange("kc p n -> p kc n"))

        def load_b1():
            nc.sync.dma_start(out=yts[1][:], in_=yin[1].rearrange("(kc p) n -> p kc n", p=128))
            nc.sync.dma_start(out=xts[1][:], in_=xin[1].rearrange("(kc p) n -> p kc n", p=128))

        # ---------------------------------------------------------------
        # Startup path for (b0, mc0): heads 0,1 run f16 S_T (no fp8
        # round-trip on the critical path). qn = raw f16 q; kn carries the
        # combined scale 1/(||q||*||k||). PSUM goes through the (still
        # unused) st ring so the acc ring can't serialize the q->k chain.
        def proj_qk_f16(nb, mc, qn, kn):
            # PE warm-up: keep the PE busy from ~t=1us so the p-state is at
            # full clock when the real projections arrive.
            wsrc = big.tile([128, 512], f16, tag="wsrc", bufs=1, name="wsrc")
            nc.gpsimd.memset(wsrc[:], 0.0)
            for i in range(8):
                wp = ps.tile([128, 512], f32, tag="acc", bufs=2, name="wp")
                nc.tensor.matmul(wp[:], wsrc[:, 0:128], wsrc[:],
                                 start=True, stop=True)
            qp = ps.tile([128, N], f32, tag="st", bufs=2, name="qp_s")
            for ih in range(2):
                for kc in range(2):
                    nc.tensor.matmul(
                        qp[:, ih * 512:(ih + 1) * 512],
                        wqk_sb[:, kc, mc * 128:(mc + 1) * 128],
                        xts[nb][:, kc, ih * 512:(ih + 1) * 512],
                        start=(kc == 0), stop=(kc == 1))
            kp = ps.tile([128, N], f32, tag="st", bufs=2, name="kp_s")
            for ih in range(2):
                for kc in range(2):
                    nc.tensor.matmul(
                        kp[:, ih * 512:(ih + 1) * 512],
                        wqk_sb[:, 2 + kc, mc * 128:(mc + 1) * 128],
                        yts[nb][:, kc, ih * 512:(ih + 1) * 512],
                        start=(kc == 0), stop=(kc == 1))
            stq = sm.tile([128, 2, 6], f32, tag="stq", bufs=4, name="stq")
            stk = sm.tile([128, 2, 6], f32, tag="stk", bufs=4, name="stk")
            for ih in range(2):
                nc.vector.bn_stats(out=stq[:, ih, :],
                                   in_=qp[:, ih * 512:(ih + 1) * 512])
            nc.vector.tensor_copy(qn[:], qp[:])
            for ih in range(2):
                nc.vector.bn_stats(out=stk[:, ih, :],
                                   in_=kp[:, ih * 512:(ih + 1) * 512])
            mvq = sm.tile([128, 2], f32, tag="mvq", bufs=4, name="mvq")
            mvk = sm.tile([128, 2], f32, tag="mvk", bufs=4, name="mvk")
            nc.vector.bn_aggr(out=mvq[:], in_=stq[:])
            nc.vector.bn_aggr(out=mvk[:], in_=stk[:])
            pqk = sm.tile([128, 1], f32, tag="pqk", bufs=4, name="pqk")
            uq = sm.tile([128, 1], f32, tag="uq", bufs=4, name="uq")
            nc.vector.scalar_tensor_tensor(uq[:], mvq[:, 0:1], mvq[:, 0:1],
                                           mvq[:, 1:2], alu.mult, alu.add)
            nc.vector.scalar_tensor_tensor(pqk[:], mvk[:, 0:1], mvk[:, 0:1],
                                           mvk[:, 1:2], alu.mult, alu.add)
            nc.vector.tensor_tensor(pqk[:], pqk[:], uq[:], alu.mult)
            sck = sm.tile([128, 1], f32, tag="sck", bufs=4, name="sck")
            _quake_rsqrt(nc.vector, sm, pqk[:], sck[:], 1.0 / float(N))
            for ih in range(2):
                nc.vector.tensor_scalar(kn[:, ih * 512:(ih + 1) * 512],
                                        kp[:, ih * 512:(ih + 1) * 512],
                                        sck[:, 0:1], None, alu.mult)

        def proj_qk_f16b(nb, mc, qn, kn):
            state = {}

            def pmm(w4, srct, key, ih):
                pp = ps.tile([128, 512], f32, tag="acc", bufs=2, name="pp")
                for kc in range(2):
                    nc.tensor.matmul(
                        pp[:],
                        wqk_sb[:, w4 + kc, mc * 128:(mc + 1) * 128],
                        srct[:, kc, ih * 512:(ih + 1) * 512],
                        start=(kc == 0), stop=(kc == 1))
                state.setdefault(key, []).append(pp)

            def qstage():
                pmm(0, xts[nb], "q", 0)
                pmm(0, xts[nb], "q", 1)
                qps = state["q"]
                for ih in range(2):
                    nc.vector.tensor_copy(qn[:, ih * 512:(ih + 1) * 512],
                                          qps[ih][:])
                stq = sm.tile([128, 2, 6], f32, tag="stq", bufs=4, name="stq")
                for ih in range(2):
                    nc.vector.bn_stats(out=stq[:, ih, :],
                                       in_=qn[:, ih * 512:(ih + 1) * 512])
                mvq = sm.tile([128, 2], f32, tag="mvq", bufs=4, name="mvq")
                nc.vector.bn_aggr(out=mvq[:], in_=stq[:])
                uq = sm.tile([128, 1], f32, tag="uq", bufs=4, name="uq")
                nc.vector.scalar_tensor_tensor(uq[:], mvq[:, 0:1], mvq[:, 0:1],
                                               mvq[:, 1:2], alu.mult, alu.add)
                state["uq"] = uq

            def kstage():
                pmm(2, yts[nb], "k", 0)
                pmm(2, yts[nb], "k", 1)
                kps = state["k"]
                knr = big.tile([128, N], f16, tag="knrb", bufs=3, name="knrb")
                for ih in range(2):
                    nc.vector.tensor_copy(knr[:, ih * 512:(ih + 1) * 512],
                                          kps[ih][:])
                stk = sm.tile([128, 2, 6], f32, tag="stk", bufs=4, name="stk")
                for ih in range(2):
                    nc.vector.bn_stats(out=stk[:, ih, :],
                                       in_=knr[:, ih * 512:(ih + 1) * 512])
                mvk = sm.tile([128, 2], f32, tag="mvk", bufs=4, name="mvk")
                nc.vector.bn_aggr(out=mvk[:], in_=stk[:])
                pqk = sm.tile([128, 1], f32, tag="pqk", bufs=4, name="pqk")
                nc.vector.scalar_tensor_tensor(pqk[:], mvk[:, 0:1], mvk[:, 0:1],
                                               mvk[:, 1:2], alu.mult, alu.add)
                nc.vector.tensor_tensor(pqk[:], pqk[:], state["uq"][:],
                                        alu.mult)
                sck = sm.tile([128, 1], f32, tag="sck", bufs=4, name="sck")
                _quake_rsqrt(nc.vector, sm, pqk[:], sck[:], 1.0 / float(N))
                # scale-multiply on Pool (SBUF f16 -> f16)
                for ih in range(2):
                    nc.gpsimd.tensor_scalar(kn[:, ih * 512:(ih + 1) * 512],
                                            knr[:, ih * 512:(ih + 1) * 512],
                                            sck[:, 0:1], None, alu.mult)

            return [qstage, kstage]

        def alloc_v(vts_nb):
            for jc in range(8):
                vt = big.tile([128, 4, 128], f16, tag="vt", bufs=18,
                              name=f"vt{jc}")
                vts_nb.append(vt)

        def proj_v(nb, jcs, vts_nb):
            for jc in jcs:
                vp = ps.tile([128, 512], f32, tag="acc", bufs=2)
                for kc in range(2):
                    nc.tensor.matmul(
                        vp[:, 0:HID],
                        yts[nb][:, kc, jc * 128:(jc + 1) * 128],
                        wvo_sb[:, kc, :],
                        start=(kc == 0), stop=(kc == 1))
                vt = vts_nb[jc]
                nc.vector.tensor_copy(vt[:, :, 0:64],
                                      vp[:, 0:HID].rearrange("p (h d) -> p h d", h=4))
                nc.gpsimd.memset(vt[:, :, 64:128], 1.0)

        # one attention head. Fillers fire between the next S_T emission and
        # the PV emission, so a PV stall (waiting on exp) lets ready filler
        # work run first and the handed-off S_T is never queued behind it.
        # Head h emits head h+1's first S_T before its own last PV.
        def make_head(nb, h, q8r2, k8r2, vts_nb, o_sb, qn=None, kn=None):
            mc, ha = h // 2, h % 2

            if qn is not None:
                def st_mm(jc):
                    st = ps.tile([128, N], f32, tag="st", bufs=2)
                    for ih in range(2):
                        nc.tensor.matmul(
                            st[:, ih * 512:(ih + 1) * 512],
                            kn[64 * ha:64 * (ha + 1), jc * 128:(jc + 1) * 128],
                            qn[64 * ha:64 * (ha + 1), ih * 512:(ih + 1) * 512],
                            start=True, stop=True)
                    return st
                scale = 10.0
            else:
                q8r, k8r = q8r2[mc], k8r2[mc]

                def st_mm(jc):
                    st = ps.tile([128, N], f32, tag="st", bufs=2)
                    for ih in range(2):
                        nc.tensor.matmul(
                            st[:, ih * 512:(ih + 1) * 512],
                            k8r[ha * 32:(ha + 1) * 32, :, jc * 128:(jc + 1) * 128],
                            q8r[ha * 32:(ha + 1) * 32, :, ih * 512:(ih + 1) * 512],
                            start=True, stop=True, perf_mode=DR)
                    return st
                scale = 10.0 / KGAIN

            return {"nb": nb, "h": h, "mc": mc, "ha": ha, "vts": vts_nb,
                    "o_sb": o_sb, "st_mm": st_mm, "first_st": None,
                    "scale": scale}

        def run_head(hc, next_hc, fillers=(), handoff=True):
            h, ha, hp = hc["h"], hc["ha"], hc["mc"]
            _mark(nc, f"head(b{hc['nb']},h{h}) start")
            hr = 64 * ha
            o_sb, vts_nb = hc["o_sb"], hc["vts"]
            fi = iter(fillers)
            op = ps.tile([128, N], f32, tag="op", bufs=1, name="op")
            sts = [hc["first_st"] if hc["first_st"] is not None
                   else hc["st_mm"](0)]
            for jc in range(8):
                et = big.tile([128, N], f16, tag="et", bufs=8, name="et")
                nc.scalar.activation(out=et[:], in_=sts[jc][:], func=EXP,
                                     scale=hc["scale"])
                if jc < 7:
                    sts.append(hc["st_mm"](jc + 1))
                elif next_hc is not None and handoff:
                    next_hc["first_st"] = next_hc["st_mm"](0)
                f = next(fi, None)
                if f is not None:
                    _mark(nc, f"head(b{hc['nb']},h{h}) filler jc{jc}")
                    f()
                    _mark(nc, f"head(b{hc['nb']},h{h}) filler jc{jc} end")
                vt = vts_nb[jc]
                for ih in range(2):
                    nc.tensor.matmul(
                        op[:, ih * 512:(ih + 1) * 512],
                        vt[:, h, :],
                        et[:, ih * 512:(ih + 1) * 512],
                        start=(jc == 0), stop=(jc == 7))
            _mark(nc, f"head(b{hc['nb']},h{h}) norm")
            for ihn in range(2):
                sl = slice(ihn * 512, (ihn + 1) * 512)
                db = big.tile([64, 512], f32, tag="db", bufs=8, name="db")
                nc.vector.reciprocal(db[:], op[64:128, sl])
                nc.vector.tensor_tensor(o_sb[hr:hr + 64, hp, sl],
                                        op[0:64, sl], db[:], alu.mult)
            for f in fi:
                _mark(nc, f"head(b{hc['nb']},h{h}) leftover")
                f()
            _mark(nc, f"head(b{hc['nb']},h{h}) end")

        def zproj(nb, o_sb, mcs=(0, 1), ihs=(0, 1)):
            for mc in mcs:
                for ih in ihs:
                    sl = slice(ih * 512, (ih + 1) * 512)
                    msl = slice(mc * 128, (mc + 1) * 128)
                    zp = ps.tile([128, 512], f32, tag="acc", bufs=2)
                    nc.tensor.matmul(zp[:], wvo_sb[:, 2, msl],
                                     o_sb[:, 0, sl], start=True, stop=False)
                    nc.tensor.matmul(zp[:], wvo_sb[:, 3, msl],
                                     o_sb[:, 1, sl], start=False, stop=True)
                    zs = big.tile([128, 512], f32, tag="zs", bufs=8)
                    nc.vector.tensor_scalar(zs[:], zp[:], b_sb[:, mc, 0:1],
                                            None, alu.add)
                    nc.sync.dma_start(
                        out=out[nb, msl, sl],
                        in_=zs[:])

        def alloc_qk():
            q8r2 = [big.tile([64, 2, N], f8, tag="q8r", bufs=4, name=f"q8r{i}")
                    for i in range(2)]
            k8r2 = [big.tile([64, 2, N], f8, tag="k8r", bufs=4, name=f"k8r{i}")
                    for i in range(2)]
            return q8r2, k8r2

        def alloc_o():
            return big.tile([128, 2, N], f16, tag="osb", bufs=2, name="osb")

        # ---- schedule -------------------------------------------------
        o0 = alloc_o()
        o1 = alloc_o()
        qns, kns = [], []
        for i in range(4):
            qns.append(big.tile([128, N], f16, tag="qn", bufs=4, name=f"qn{i}"))
            kns.append(big.tile([128, N], f16, tag="kn", bufs=4, name=f"kn{i}"))
        vts0, vts1 = [], []
        proj_qk_f16(0, 0, qns[0], kns[0])
        with tc.tile_wait_until(0.0072):
            wsrc2 = big.tile([128, 512], f16, tag="wsrc", bufs=1, name="wsrc2")
            nc.gpsimd.memset(wsrc2[:], 0.0)
            for i in range(9):
                wp2 = ps.tile([128, 512], f32, tag="acc", bufs=2, name="wp2")
                nc.tensor.matmul(wp2[:], wsrc2[:, 0:128], wsrc2[:],
                                 start=True, stop=True)
        with tc.tile_wait_until(0.004):
            load_b1()
        qk01 = proj_qk_f16b(0, 1, qns[1], kns[1])
        qk10 = proj_qk_f16b(1, 0, qns[2], kns[2])
        qk11 = proj_qk_f16b(1, 1, qns[3], kns[3])
        alloc_v(vts0)
        alloc_v(vts1)
        with tc.tile_wait_until(0.013):
            qk01[0](); qk01[1]()
        with tc.tile_wait_until(0.014):
            proj_v(0, range(4), vts0)
        with tc.tile_wait_until(0.019):
            proj_v(0, range(4, 8), vts0)
        with tc.tile_wait_until(0.022):
            qk10[0](); qk10[1]()
        with tc.tile_wait_until(0.031):
            qk11[0](); qk11[1]()
        hcs = [make_head(0, 0, None, None, vts0, o0, qns[0], kns[0]),
               make_head(0, 1, None, None, vts0, o0, qns[0], kns[0]),
               make_head(0, 2, None, None, vts0, o0, qns[1], kns[1]),
               make_head(0, 3, None, None, vts0, o0, qns[1], kns[1]),
               make_head(1, 0, None, None, vts1, o1, qns[2], kns[2]),
               make_head(1, 1, None, None, vts1, o1, qns[2], kns[2]),
               make_head(1, 2, None, None, vts1, o1, qns[3], kns[3]),
               make_head(1, 3, None, None, vts1, o1, qns[3], kns[3])]
        run_head(hcs[0], hcs[1])
        run_head(hcs[1], hcs[2])
        with tc.tile_wait_until(0.030):
            proj_v(1, range(4), vts1)
        with tc.tile_wait_until(0.034):
            proj_v(1, range(4, 8), vts1)
        run_head(hcs[2], hcs[3])
        run_head(hcs[3], hcs[4])
        with tc.tile_wait_until(0.041):
            zproj(0, o0)
        run_head(hcs[4], hcs[5])
        run_head(hcs[5], hcs[6])
        run_head(hcs[6], hcs[7])
        run_head(hcs[7], None)
        zproj(1, o1)

    nc.finalize()
    return nc


def _get_nc():
    if "nc" not in _CACHE:
        _CACHE["nc"] = _build_nc()
    return _CACHE["nc"]


def kernel(x, y, w_qkv, w_out, b_out):
    from concourse.bass_utils import run_bass_kernel_spmd

    nc = _get_nc()

    x = np.asarray(x, dtype=np.float32).reshape(16, C, N).astype(np.float16)
    y = np.asarray(y, dtype=np.float32).reshape(16, C, N).astype(np.float16)
    w_qkv = np.asarray(w_qkv, dtype=np.float32)
    wq_t = np.ascontiguousarray(w_qkv[0:HID].T).astype(np.float16)
    wk_t = np.ascontiguousarray(w_qkv[HID:2 * HID].T).astype(np.float16)
    wv_t = np.ascontiguousarray(w_qkv[2 * HID:3 * HID].T).astype(np.float16)
    wo_t = np.ascontiguousarray(np.asarray(w_out, dtype=np.float32).T).astype(np.float16)
    bo = np.ascontiguousarray(
        np.asarray(b_out, dtype=np.float32).reshape(2, 128, 1))

    def pack2(a, b):
        # [128, 4, 256]: [:, 0:2] = a chunks, [:, 2:4] = b chunks, where
        # [:, w*2+kc, n] = w_t[kc*128 + p, n]
        pk = np.empty((128, 4, HID), dtype=np.float16)
        pk[:, 0:2] = a.reshape(2, 128, HID).transpose(1, 0, 2)
        pk[:, 2:4] = b.reshape(2, 128, HID).transpose(1, 0, 2)
        return pk

    wqk = pack2(wq_t, wk_t)
    wvo = pack2(wv_t, wo_t)

    in_maps = []
    for c in range(NCORES):
        in_maps.append({
            "x": np.ascontiguousarray(x[c * NB:(c + 1) * NB]),
            "y": np.ascontiguousarray(y[c * NB:(c + 1) * NB]),
            "wqk": wqk, "wvo": wvo,
            "b_out": bo,
        })

    res = run_bass_kernel_spmd(nc, in_maps, list(range(NCORES)))
    full = np.concatenate([res.results[i]["out"] for i in range(NCORES)], axis=0)
    return full.reshape(16, C, 32, 32)



# revision 23
# speedup vs baseline: 1.1683x; 1.1683x over previous
"""Cross-attention kernel for 8 trn2 NeuronCores.

Reference computation (per batch b of 16):
  q = Wq @ x, k = Wk @ y, v = Wv @ y          (1x1 convs as channel matmuls)
  q,k l2-normalized over the SPATIAL axis (per (h,d) row)
  sim = 10 * q^T k per head; attn = softmax_j(sim); o = attn @ v^T
  out = Wo @ o + b

Sharding: data-parallel over batch, 2 batches per core, weights replicated.

v4 design (ACT/PE/DVE co-roofline, built against the TimelineSim model):
  - S_T (q^T k) in fp8e4m3 + DoubleRow for heads 2..7: q raw (N(0,1) fits
    e4m3), k carries the combined l2 scale sq*sk*1024.  The DR pair layout
    [32p, 2pair, n] is produced WITHOUT a DRAM round-trip: the host permutes
    the Wq/Wk output columns so the projection PSUM partitions come out as
    [pair, ha, dlow]; two partition-base-offset copies then write the
    [64, 2, n] pair tile directly.
  - Heads 0-1 (first batch, mc0) use f16 S_T with unpermuted weights so the
    first exp starts ~8.5us (no quantize on the critical path); their qn
    copies run on the (otherwise idle) ACT engine.
  - Softmax normalize in ONE DVE op: the PV ones-block and Wv are pre-scaled
    by R0~=1/1027, so den*R0 ~= 1+-0.01 and one Newton step from the
    constant seed is exact to ~1e-4:  o = (den*R0 - 2) * (R0*EV) = -o_true,
    with the sign folded into Wo on the host.
  - All projection/quantize/v/zproj work is drip-fed into the 64 exp slots
    via per-jc fillers so the in-order PE queue never delays the next S_T.
  - Tail: zproj pieces run immediately per (mc, ih); the nb=1 bias-adds ride
    the ACT engine (Identity+bias, same act table as Exp); output DMA is f16.
  - PSUM: st [128,1024]f32 x2 (4 banks) + op [128,1024] x1 (2) + acc
    [128,512] x2 (2).
"""

import sys
from collections import deque

import numpy as np

if "/opt/trn_rl_repo" not in sys.path:
    sys.path.insert(0, "/opt/trn_rl_repo")

NB = 2        # batches per core
C = 256       # channels
N = 1024      # spatial (32*32)
HEADS = 4
DH = 64
HID = 256
NCORES = 8
MAGIC = 0x5F3759DF  # Quake fast inverse-sqrt seed
KGAIN = 1024.0      # power-of-two gain folded into k8; exp scale = 10/KGAIN
R0 = 1.0 / 1027.0   # Newton seed for 1/den (den ~= 1024 * (1 + E[s^2]/2))

_CACHE = {}
PHASES = []


def _mark(nc, label):
    PHASES.append((int(nc.get_next_instruction_name()[2:]), label))


def _quake_rsqrt(eng, pool, p_ap, out_ap, final_scale, iters=2):
    """out = rsqrt(p) * final_scale for [128,1] fp32 APs on engine `eng`.

    Quake seed + Newton iterations (1 iter: rel err ~2e-3; 2: ~1e-6).
    """
    from concourse import mybir

    i32 = mybir.dt.int32
    alu = mybir.AluOpType
    t = pool.tile([128, 1], mybir.dt.float32, tag="qk_rs_t", bufs=4)
    r = pool.tile([128, 1], mybir.dt.float32, tag="qk_rs_r", bufs=4)
    a = pool.tile([128, 1], mybir.dt.float32, tag="qk_rs_a", bufs=4)
    eng.tensor_scalar(t.bitcast(i32), p_ap.bitcast(i32), 1, None,
                      alu.logical_shift_right)
    eng.tensor_scalar(r.bitcast(i32), t.bitcast(i32), -1, MAGIC,
                      alu.mult, alu.add)
    cur = r
    if iters == 2:
        eng.scalar_tensor_tensor(a[:], r[:], r[:, 0:1], p_ap,
                                 alu.mult, alu.mult)
        eng.tensor_scalar(a[:], a[:], -0.5, 1.5, alu.mult, alu.add)
        eng.tensor_scalar(t[:], a[:], r[:, 0:1], None, alu.mult)
        cur = t
    eng.scalar_tensor_tensor(a[:], cur[:], cur[:, 0:1], p_ap,
                             alu.mult, alu.mult)
    eng.tensor_scalar(a[:], a[:], -0.5, 1.5, alu.mult, alu.add)
    eng.tensor_scalar(out_ap, a[:], cur[:, 0:1], final_scale,
                      alu.mult, alu.mult)


def _build_nc():
    from contextlib import ExitStack

    import concourse.tile as tile
    from concourse import bacc, mybir

    f32 = mybir.dt.float32
    f16 = mybir.dt.float16
    f8 = mybir.dt.float8e4
    alu = mybir.AluOpType
    EXP = mybir.ActivationFunctionType.Exp
    COPY = mybir.ActivationFunctionType.Copy
    IDENT = mybir.ActivationFunctionType.Identity
    DR = mybir.MatmulPerfMode.DoubleRow

    nc = bacc.Bacc("TRN2", target_bir_lowering=False)

    xin = nc.dram_tensor("x", [NB, C, N], f16, kind="ExternalInput")
    yin = nc.dram_tensor("y", [NB, C, N], f16, kind="ExternalInput")
    # wqk slots (s_qk below): [0:4] = startup mc0-unperm q/k x kc (loaded
    # first, small DMA); [4:8] = q mc0-perm/mc1-perm; [8:12] = k perm.
    wqk = nc.dram_tensor("wqk", [128, 12, 128], f16, kind="ExternalInput")
    # wvo slots: [kc0 wv, kc1 wv, kc0 wo, kc1 wo]; wv scaled by R0, wo by -1.
    wvo = nc.dram_tensor("wvo", [128, 4, HID], f16, kind="ExternalInput")
    bo = nc.dram_tensor("b_out", [2, 128, 1], f32, kind="ExternalInput")
    out = nc.dram_tensor("out", [NB, C, N], f16, kind="ExternalOutput")

    with tile.TileContext(nc) as tc, ExitStack() as ctx:
        consts = ctx.enter_context(tc.tile_pool(name="consts", bufs=1))
        big = ctx.enter_context(tc.tile_pool(name="big", bufs=2))
        sm = ctx.enter_context(tc.tile_pool(name="sm", bufs=4))
        ps = ctx.enter_context(tc.tile_pool(name="ps", bufs=2, space="PSUM"))

        # ---- constants + input DMA ------------------------------------
        wqk_sb = consts.tile([128, 12, 128], f16, tag="wqk")
        wvo_sb = consts.tile([128, 4, HID], f16, tag="wvo")
        b_sb = consts.tile([128, 2, 1], f32, tag="bo")
        xts, yts = [], []
        for nb in range(NB):
            xts.append(big.tile([128, 2, N], f16, tag="xt", bufs=2,
                                name=f"xt{nb}"))
            yts.append(big.tile([128, 2, N], f16, tag="yt", bufs=2,
                                name=f"yt{nb}"))
        # warm the ACT exp table while input DMAs are in flight
        warm = sm.tile([128, 1], f32, tag="warm", bufs=1)
        nc.vector.memset(warm[:], 0.0)
        nc.scalar.activation(out=warm[:], in_=warm[:], func=EXP, scale=1.0)
        # startup-critical loads first (k-side before q-side), column-split
        # so the first projection matmuls start one DMA earlier.
        nc.sync.dma_start(out=wqk_sb[:, 0:4, :], in_=wqk[:, 0:4, :])
        yr0 = yin[0].rearrange("(kc p) n -> p kc n", p=128)
        xr0 = xin[0].rearrange("(kc p) n -> p kc n", p=128)
        nc.sync.dma_start(out=yts[0][:, :, 0:512], in_=yr0[:, :, 0:512])
        nc.sync.dma_start(out=yts[0][:, :, 512:1024], in_=yr0[:, :, 512:1024])
        nc.sync.dma_start(out=xts[0][:, :, 0:512], in_=xr0[:, :, 0:512])
        nc.sync.dma_start(out=xts[0][:, :, 512:1024], in_=xr0[:, :, 512:1024])
        nc.sync.dma_start(out=wvo_sb[:], in_=wvo[:])
        nc.sync.dma_start(out=wqk_sb[:, 4:12, :], in_=wqk[:, 4:12, :])
        nc.sync.dma_start(out=b_sb[:], in_=bo.rearrange("kc p n -> p kc n"))

        # ---- PE p-state warmup (rotating acc tiles; runs in DMA wait) -
        wsrc = big.tile([128, 512], f16, tag="wsrc", bufs=1, name="wsrc")
        nc.gpsimd.memset(wsrc[:], 0.0)
        for _ in range(7):
            wp = ps.tile([128, 512], f32, tag="acc", bufs=2, name="wp")
            nc.tensor.matmul(wp[:], wsrc[:, 0:128], wsrc[:],
                             start=True, stop=True)

        # ---- persistent attention tiles -------------------------------
        qn = big.tile([128, N], f16, tag="qn", bufs=1, name="qn")
        kn = big.tile([128, N], f16, tag="kn", bufs=1, name="kn")
        q8s, k8s = {}, {}
        for key in ((0, 1), (1, 0), (1, 1)):
            q8s[key] = big.tile([64, 2, N], f8, tag="q8", bufs=3,
                                name=f"q8_{key[0]}{key[1]}")
            k8s[key] = big.tile([64, 2, N], f8, tag="k8", bufs=3,
                                name=f"k8_{key[0]}{key[1]}")
        o0 = big.tile([128, 2, N], f16, tag="osb", bufs=2, name="o0")
        o1 = big.tile([128, 2, N], f16, tag="osb", bufs=2, name="o1")
        vts0 = [big.tile([128, 4, 128], f16, tag="vt", bufs=16,
                         name=f"vt0_{jc}") for jc in range(8)]
        vts1 = [big.tile([128, 4, 128], f16, tag="vt", bufs=16,
                         name=f"vt1_{jc}") for jc in range(8)]

        QK_SLOT = {(1, 0, 0): 0, (1, 0, 1): 1, (0, 0, 0): 2, (0, 0, 1): 3,
                   (0, 1, 0): 4, (0, 1, 1): 5, (0, 2, 0): 6, (0, 2, 1): 7,
                   (1, 1, 0): 8, (1, 1, 1): 9, (1, 2, 0): 10, (1, 2, 1): 11}

        def s_qk(w, chunk, kc):
            return QK_SLOT[(w, chunk, kc)]

        # ---- (0,0) f16 startup chain ----------------------------------
        # qp lives in the two acc halves (its readers -- stats + the ACT qn
        # copies -- are off the k-side critical chain); kp lives in the
        # until-ST0-idle st pool so it never waits on the qn copies.
        _mark(nc, "startup chain")
        # k projection first (y loads first); kp in the until-ST0-idle st
        # pool so it never waits on the qn ACT copies.
        kp = ps.tile([128, N], f32, tag="st", bufs=2, name="kp00")
        for ih in range(2):
            for kc in range(2):
                nc.tensor.matmul(kp[:, ih * 512:(ih + 1) * 512],
                                 wqk_sb[:, s_qk(1, 0, kc), :],
                                 yts[0][:, kc, ih * 512:(ih + 1) * 512],
                                 start=(kc == 0), stop=(kc == 1))
        stk = sm.tile([128, 2, 6], f32, tag="stk", bufs=4, name="stk00")
        nc.vector.bn_stats(out=stk[:, 0, :], in_=kp[:, 0:512])
        qph = []
        for ih in range(2):
            qp = ps.tile([128, 512], f32, tag="acc", bufs=2, name=f"qp00_{ih}")
            for kc in range(2):
                nc.tensor.matmul(qp[:], wqk_sb[:, s_qk(0, 0, kc), :],
                                 xts[0][:, kc, ih * 512:(ih + 1) * 512],
                                 start=(kc == 0), stop=(kc == 1))
            qph.append(qp)
        nc.vector.bn_stats(out=stk[:, 1, :], in_=kp[:, 512:1024])
        stq = sm.tile([128, 2, 6], f32, tag="stq", bufs=4, name="stq00")
        for ih in range(2):
            nc.vector.bn_stats(out=stq[:, ih, :], in_=qph[ih][:])
        # qn head-0 copies on ACT (idle until first exp); head-1 copies on
        # DVE (needed only by head 1, ~8us later) so they never preempt exps
        for ih in range(2):
            nc.scalar.activation(
                out=qn[0:64, ih * 512:(ih + 1) * 512],
                in_=qph[ih][0:64, :], func=COPY, scale=1.0)
        with tc.tile_wait_until(0.012):
            for ih in range(2):
                nc.vector.tensor_copy(
                    qn[64:128, ih * 512:(ih + 1) * 512],
                    qph[ih][64:128, :])
        mvq = sm.tile([128, 2], f32, tag="mvq", bufs=4, name="mvq00")
        mvk = sm.tile([128, 2], f32, tag="mvk", bufs=4, name="mvk00")
        nc.vector.bn_aggr(out=mvq[:], in_=stq[:])
        nc.vector.bn_aggr(out=mvk[:], in_=stk[:])
        uq = sm.tile([128, 1], f32, tag="uq", bufs=4, name="uq00")
        pqk = sm.tile([128, 1], f32, tag="pqk", bufs=4, name="pqk00")
        nc.vector.scalar_tensor_tensor(uq[:], mvq[:, 0:1], mvq[:, 0:1],
                                       mvq[:, 1:2], alu.mult, alu.add)
        nc.vector.scalar_tensor_tensor(pqk[:], mvk[:, 0:1], mvk[:, 0:1],
                                       mvk[:, 1:2], alu.mult, alu.add)
        nc.vector.tensor_tensor(pqk[:], pqk[:], uq[:], alu.mult)
        sck16 = sm.tile([128, 1], f32, tag="sck", bufs=4, name="sck00")
        _quake_rsqrt(nc.vector, sm, pqk[:], sck16[:], 1.0 / float(N),
                     iters=1)
        # kn scale: jc0 block first so the first S_T can fire, then the rest
        nc.vector.tensor_scalar(kn[:, 0:128], kp[:, 0:128],
                                sck16[:, 0:1], None, alu.mult)

        # f16 S_T for heads 0-1
        def st_mm_f16(ha):
            def mm(jc):
                st = ps.tile([128, N], f32, tag="st", bufs=2)
                for ih in range(2):
                    nc.tensor.matmul(
                        st[:, ih * 512:(ih + 1) * 512],
                        kn[64 * ha:64 * (ha + 1), jc * 128:(jc + 1) * 128],
                        qn[64 * ha:64 * (ha + 1), ih * 512:(ih + 1) * 512],
                        start=True, stop=True)
                return st
            return mm

        st00 = st_mm_f16(0)
        first_st_00 = st00(0)
        nc.vector.tensor_scalar(kn[:, 128:512], kp[:, 128:512],
                                sck16[:, 0:1], None, alu.mult)
        nc.vector.tensor_scalar(kn[:, 512:1024], kp[:, 512:1024],
                                sck16[:, 0:1], None, alu.mult)

        # ---- startup v-projection (jc 0..3) through the op-pool tile --
        vpb = ps.tile([128, N], f32, tag="op", bufs=1, name="vpb")
        for jc in range(4):
            for kc in range(2):
                nc.tensor.matmul(vpb[:, jc * 256:(jc + 1) * 256],
                                 yts[0][:, kc, jc * 128:(jc + 1) * 128],
                                 wvo_sb[:, kc, :],
                                 start=(kc == 0), stop=(kc == 1))
        # gate the vt copies past the startup DVE chain (~9.5us) so the
        # readiness-greedy scheduler can't interleave them into it
        with tc.tile_wait_until(0.0095):
            for jc in range(4):
                nc.vector.tensor_copy(
                    vts0[jc][:, :, 0:64],
                    vpb[:, jc * 256:(jc + 1) * 256].rearrange("p (h d) -> p h d", h=4))
                nc.gpsimd.memset(vts0[jc][:, :, 64:128], R0)

        # ---- filler pieces --------------------------------------------
        def load_b1():
            nc.sync.dma_start(out=yts[1][:], in_=yin[1].rearrange("(kc p) n -> p kc n", p=128))
            nc.sync.dma_start(out=xts[1][:], in_=xin[1].rearrange("(kc p) n -> p kc n", p=128))

        def proj_v_piece(nb, jc, vts_nb):
            vp = ps.tile([128, 512], f32, tag="acc", bufs=2)
            for kc in range(2):
                nc.tensor.matmul(vp[:, 0:HID],
                                 yts[nb][:, kc, jc * 128:(jc + 1) * 128],
                                 wvo_sb[:, kc, :],
                                 start=(kc == 0), stop=(kc == 1))
            nc.vector.tensor_copy(
                vts_nb[jc][:, :, 0:64],
                vp[:, 0:HID].rearrange("p (h d) -> p h d", h=4))
            nc.gpsimd.memset(vts_nb[jc][:, :, 64:128], R0)

        def fp8_proj_pieces(nb, mc, q8t, k8t):
            """10 filler closures: project q/k (permuted cols), l2 stats,
            quantize into DR pair tiles."""
            st_ = {}
            tag = f"{nb}{mc}"
            chunk = 1 if mc == 0 else 2

            def qmm(ih):
                def f():
                    qp = ps.tile([128, 512], f32, tag="acc", bufs=2)
                    for kc in range(2):
                        nc.tensor.matmul(qp[:], wqk_sb[:, s_qk(0, chunk, kc), :],
                                         xts[nb][:, kc, ih * 512:(ih + 1) * 512],
                                         start=(kc == 0), stop=(kc == 1))
                    st_[f"qp{ih}"] = qp
                    if ih == 0:
                        st_["stq"] = sm.tile([128, 2, 6], f32, tag="stq",
                                             bufs=4, name=f"stq{tag}")
                    else:
                        nc.vector.bn_stats(out=st_["stq"][:, 0, :],
                                           in_=st_["qp0"][:])
                return f

            def qfin(ih):
                def f():
                    qp = st_[f"qp{ih}"]
                    if ih == 1:
                        nc.vector.bn_stats(out=st_["stq"][:, 1, :], in_=qp[:])
                    for pair in range(2):
                        nc.vector.tensor_copy(
                            q8t[:, pair, ih * 512:(ih + 1) * 512],
                            qp[64 * pair:64 * (pair + 1), :])
                return f

            def kmm(ih):
                def f():
                    kp = ps.tile([128, 512], f32, tag="acc", bufs=2)
                    for kc in range(2):
                        nc.tensor.matmul(kp[:], wqk_sb[:, s_qk(1, chunk, kc), :],
                                         yts[nb][:, kc, ih * 512:(ih + 1) * 512],
                                         start=(kc == 0), stop=(kc == 1))
                    st_[f"kp{ih}"] = kp
                    if ih == 0:
                        st_["stk"] = sm.tile([128, 2, 6], f32, tag="stk",
                                             bufs=4, name=f"stk{tag}")
                    else:
                        nc.vector.bn_stats(out=st_["stk"][:, 0, :],
                                           in_=st_["kp0"][:])
                return f

            def kcomb():
                nc.vector.bn_stats(out=st_["stk"][:, 1, :], in_=st_["kp1"][:])
                mvq_ = sm.tile([128, 2], f32, tag="mvq", bufs=4)
                mvk_ = sm.tile([128, 2], f32, tag="mvk", bufs=4)
                nc.vector.bn_aggr(out=mvq_[:], in_=st_["stq"][:])
                nc.vector.bn_aggr(out=mvk_[:], in_=st_["stk"][:])
                uq_ = sm.tile([128, 1], f32, tag="uq", bufs=4)
                pqk_ = sm.tile([128, 1], f32, tag="pqk", bufs=4)
                nc.vector.scalar_tensor_tensor(uq_[:], mvq_[:, 0:1], mvq_[:, 0:1],
                                               mvq_[:, 1:2], alu.mult, alu.add)
                nc.vector.scalar_tensor_tensor(pqk_[:], mvk_[:, 0:1], mvk_[:, 0:1],
                                               mvk_[:, 1:2], alu.mult, alu.add)
                nc.vector.tensor_tensor(pqk_[:], pqk_[:], uq_[:], alu.mult)
                sck = sm.tile([128, 1], f32, tag="sck", bufs=4, name=f"sck{tag}")
                # k8 = kp * sck * KGAIN; KGAIN/N == 1.0
                _quake_rsqrt(nc.vector, sm, pqk_[:], sck[:], KGAIN / float(N),
                             iters=1)
                st_["sck"] = sck

            def kfin(ih):
                def f():
                    kp = st_[f"kp{ih}"]
                    sck = st_["sck"]
                    for pair in range(2):
                        nc.vector.tensor_scalar(
                            k8t[:, pair, ih * 512:(ih + 1) * 512],
                            kp[64 * pair:64 * (pair + 1), :],
                            sck[64 * pair:64 * (pair + 1), 0:1], None, alu.mult)
                return f

            return [qmm(0), qmm(1), qfin(0), qfin(1),
                    kmm(0), kmm(1), kcomb, kfin(0), kfin(1)]

        def zproj_piece(nb, o_sb, mc, ih, on_act):
            def f():
                sl = slice(ih * 512, (ih + 1) * 512)
                msl = slice(mc * 128, (mc + 1) * 128)
                zp = ps.tile([128, 512], f32, tag="acc", bufs=2)
                nc.tensor.matmul(zp[:], wvo_sb[:, 2, msl], o_sb[:, 0, sl],
                                 start=True, stop=False)
                nc.tensor.matmul(zp[:], wvo_sb[:, 3, msl], o_sb[:, 1, sl],
                                 start=False, stop=True)
                zs = big.tile([128, 512], f16, tag="zs", bufs=4)
                if on_act:
                    nc.scalar.activation(out=zs[:], in_=zp[:], func=IDENT,
                                         bias=b_sb[:, mc, 0:1], scale=1.0)
                else:
                    nc.vector.tensor_scalar(zs[:], zp[:], b_sb[:, mc, 0:1],
                                            None, alu.add)
                nc.sync.dma_start(out=out[nb, msl, sl], in_=zs[:])
            return f

        # ---- heads ----------------------------------------------------
        def st_mm_fp8(nb, mc, ha):
            q8t, k8t = q8s[(nb, mc)], k8s[(nb, mc)]

            def mm(jc):
                st = ps.tile([128, N], f32, tag="st", bufs=2)
                for ih in range(2):
                    nc.tensor.matmul(
                        st[:, ih * 512:(ih + 1) * 512],
                        k8t[ha * 32:(ha + 1) * 32, :, jc * 128:(jc + 1) * 128],
                        q8t[ha * 32:(ha + 1) * 32, :, ih * 512:(ih + 1) * 512],
                        start=True, stop=True, perf_mode=DR)
                return st
            return mm

        made = []

        def make_head(nb, h, vts_nb, o_sb, f16_path=False):
            mc, ha = h // 2, h % 2
            if f16_path:
                st_mm = st_mm_f16(ha)
                scale = 10.0
            else:
                st_mm = st_mm_fp8(nb, mc, ha)
                scale = 10.0 / KGAIN
            hc = {"nb": nb, "h": h, "mc": mc, "ha": ha, "vts": vts_nb,
                  "o_sb": o_sb, "st_mm": st_mm, "first_st": None,
                  "scale": scale, "idx": len(made)}
            made.append(hc)
            return hc

        fillers = deque()

        def run_head(hc, next_hc):
            h = hc["h"]
            _mark(nc, f"head(b{hc['nb']},h{h}) start")
            op = ps.tile([128, N], f32, tag="op", bufs=1, name="op")
            sts = [hc["first_st"] if hc["first_st"] is not None
                   else hc["st_mm"](0)]
            deferred_pv = []
            quad_jcs = {q[1] for q in QUAD if q[0] == hc["idx"]}
            last_jc = 7 if 7 not in quad_jcs else max(
                j for j in range(8) if j not in quad_jcs)
            for jc in range(8):
                et = big.tile([128, N], f16, tag="et", bufs=8, name="et")
                if hc.get("tail") and jc == 7:
                    # split the last exp per-ih so the tail PV/norm/zproj
                    # pipeline starts half an exp earlier
                    for ih in range(2):
                        sl = slice(ih * 512, (ih + 1) * 512)
                        nc.scalar.activation(out=et[:, sl], in_=sts[jc][:, sl],
                                             func=EXP, scale=hc["scale"])
                elif (hc["idx"], jc) in QUAD:
                    # quadratic exp offload: e ~= 1 + u + u^2/2, u = scale*st
                    # s16 on DVE (PSUM read), the polynomial on Pool/DVE f16
                    a = hc["scale"]
                    s16 = big.tile([128, N], f16, tag="s16", bufs=2,
                                   name="s16")
                    t2 = big.tile([128, N], f16, tag="t2", bufs=2, name="t2")
                    nc.vector.tensor_copy(s16[:], sts[jc][:])
                    nc.gpsimd.tensor_scalar(t2[:], s16[:], 2.0 / a, None,
                                            alu.add)
                    nc.gpsimd.tensor_tensor(t2[:], t2[:], s16[:], alu.mult)
                    nc.gpsimd.tensor_scalar(et[:], t2[:], a * a / 2.0, 1.0,
                                            alu.mult, alu.add)
                else:
                    nc.scalar.activation(out=et[:], in_=sts[jc][:], func=EXP,
                                         scale=hc["scale"])
                if jc < 7:
                    sts.append(hc["st_mm"](jc + 1))
                if jc == 6 and next_hc is not None:
                    next_hc["first_st"] = next_hc["st_mm"](0)
                if fillers and jc >= 1:
                    _mark(nc, f"head(b{hc['nb']},h{h}) filler jc{jc}")
                    fillers.popleft()()
                vt = hc["vts"][jc]
                if jc in quad_jcs:
                    # quad et arrives ~5us late (Pool chain); emit its PV at
                    # the END of the accumulation group (order is free) so it
                    # never blocks the other PVs
                    def pv(jc=jc, et=et, vt=vt):
                        for ih in range(2):
                            nc.tensor.matmul(
                                op[:, ih * 512:(ih + 1) * 512],
                                vt[:, h, :], et[:, ih * 512:(ih + 1) * 512],
                                start=False, stop=True)
                    deferred_pv.append(pv)
                    continue
                for ih in range(2):
                    nc.tensor.matmul(
                        op[:, ih * 512:(ih + 1) * 512],
                        vt[:, h, :],
                        et[:, ih * 512:(ih + 1) * 512],
                        start=(jc == 0), stop=(jc == last_jc and
                                               not deferred_pv))
            for pv in deferred_pv:
                pv()
            # Newton normalize (hw allows only one PSUM read per op):
            # one [128,1024] f16 copy lands EV + den rows in SBUF (DVE cost
            # is partition-blind), Pool computes 2-den*R0, DVE multiplies at
            # f16 2x: o = (2 - den*R0) * (R0*EV) ~= o_true.
            hr = 64 * hc["ha"]
            ihs = ((0, 1),) if not hc.get("tail") else ((0,), (1,))
            for grp in ihs:
                sl = slice(grp[0] * 512, (grp[-1] + 1) * 512)
                w = len(grp) * 512
                ocp = big.tile([128, N], f16, tag="ocp", bufs=2, name="ocp")
                d2 = big.tile([64, N], f16, tag="d2", bufs=2, name="d2")
                with tc.high_priority():
                    nc.vector.tensor_copy(ocp[:, sl], op[:, sl])
                nc.vector.tensor_scalar(d2[:, sl], ocp[64:128, sl],
                                        2.0, -1.0, alu.subtract, alu.mult)
                nc.vector.tensor_tensor(
                    hc["o_sb"][hr:hr + 64, hc["mc"], sl],
                    ocp[0:64, sl], d2[:, sl], alu.mult)
            _mark(nc, f"head(b{hc['nb']},h{h}) end")

        # quadratic-exp offload slots (head idx, jc): back half, where
        # DVE/Pool have slack
        QUAD = set()
        hcs = [make_head(0, 0, vts0, o0, f16_path=True),
               make_head(0, 1, vts0, o0, f16_path=True),
               make_head(0, 2, vts0, o0),
               make_head(0, 3, vts0, o0),
               make_head(1, 0, vts1, o1),
               make_head(1, 1, vts1, o1),
               make_head(1, 2, vts1, o1),
               make_head(1, 3, vts1, o1)]
        hcs[7]["tail"] = True
        hcs[0]["first_st"] = first_st_00

        # filler order tuned against per-head deadlines: each fp8 proj's
        # quantize must land before its head pair starts; v(nb) tiles before
        # that batch's PVs reach them.
        p01 = fp8_proj_pieces(0, 1, q8s[(0, 1)], k8s[(0, 1)])
        p10 = fp8_proj_pieces(1, 0, q8s[(1, 0)], k8s[(1, 0)])
        p11 = fp8_proj_pieces(1, 1, q8s[(1, 1)], k8s[(1, 1)])
        vp0 = [lambda jc=jc: proj_v_piece(0, jc, vts0) for jc in range(4, 8)]
        vp1 = [lambda jc=jc: proj_v_piece(1, jc, vts1) for jc in range(8)]
        z0 = [zproj_piece(0, o0, mc, ih, on_act=False)
              for mc in range(2) for ih in range(2)]
        fillers.append(load_b1)
        fillers.extend([p01[0], vp0[0], p01[1], vp0[1], p01[2], vp0[2],
                        p01[3], vp0[3]])
        fillers.extend(p01[4:9])
        fillers.extend(p10)
        fillers.extend([p11[0], vp1[0], vp1[1], p11[1], vp1[2], vp1[3],
                        p11[2], vp1[4], vp1[5], p11[3], vp1[6], vp1[7]])
        fillers.extend(p11[4:9])
        fillers.extend(z0)

        for i in range(8):
            run_head(hcs[i], hcs[i + 1] if i < 7 else None)
        # drain leftover fillers (shouldn't happen), then tail zproj(1):
        # per-ih shared staging tile, bias-add split DVE/ACT, one DMA per ih
        while fillers:
            fillers.popleft()()
        for ih in range(2):
            sl = slice(ih * 512, (ih + 1) * 512)
            zs2 = big.tile([128, 2, 512], f16, tag="zs2", bufs=2, name="zs2")
            for mc in range(2):
                msl = slice(mc * 128, (mc + 1) * 128)
                zp = ps.tile([128, 512], f32, tag="acc", bufs=2, name="zp")
                nc.tensor.matmul(zp[:], wvo_sb[:, 2, msl], o1[:, 0, sl],
                                 start=True, stop=False)
                nc.tensor.matmul(zp[:], wvo_sb[:, 3, msl], o1[:, 1, sl],
                                 start=False, stop=True)
                if mc == 1:
                    nc.scalar.activation(out=zs2[:, mc, :], in_=zp[:],
                                         func=IDENT, bias=b_sb[:, mc, 0:1],
                                         scale=1.0)
                else:
                    nc.vector.tensor_scalar(zs2[:, mc, :], zp[:],
                                            b_sb[:, mc, 0:1], None, alu.add)
            nc.sync.dma_start(
                out=out[1].rearrange("(kc p) n -> p kc n", p=128)[:, :, sl],
                in_=zs2[:])

    nc.finalize()
    return nc


def _get_nc():
    if "nc" not in _CACHE:
        _CACHE["nc"] = _build_nc()
    return _CACHE["nc"]


def _perm128():
    """Permutation for DR pair layout: PSUM partition p holds output channel
    perm(p) = ha*64 + dlow + 32*pair with ha=(p%64)//32, dlow=p%32, pair=p//64."""
    p = np.arange(128)
    return ((p % 64) // 32) * 64 + (p % 32) + 32 * (p // 64)


def kernel(x, y, w_qkv, w_out, b_out):
    from concourse.bass_utils import run_bass_kernel_spmd

    nc = _get_nc()

    x = np.asarray(x, dtype=np.float32).reshape(16, C, N).astype(np.float16)
    y = np.asarray(y, dtype=np.float32).reshape(16, C, N).astype(np.float16)
    w_qkv = np.asarray(w_qkv, dtype=np.float32)
    wq_t = np.ascontiguousarray(w_qkv[0:HID].T)          # [256 c, 256 out]
    wk_t = np.ascontiguousarray(w_qkv[HID:2 * HID].T)
    wv_t = np.ascontiguousarray(w_qkv[2 * HID:3 * HID].T)
    wo_t = np.ascontiguousarray(np.asarray(w_out, dtype=np.float32).T)
    bo = np.ascontiguousarray(
        np.asarray(b_out, dtype=np.float32).reshape(2, 128, 1))

    QK_SLOT = {(1, 0, 0): 0, (1, 0, 1): 1, (0, 0, 0): 2, (0, 0, 1): 3,
               (0, 1, 0): 4, (0, 1, 1): 5, (0, 2, 0): 6, (0, 2, 1): 7,
               (1, 1, 0): 8, (1, 1, 1): 9, (1, 2, 0): 10, (1, 2, 1): 11}
    perm = _perm128()
    wqk = np.empty((128, 12, 128), dtype=np.float16)
    for w_i, wt in ((0, wq_t), (1, wk_t)):
        chunks = [wt[:, 0:128],                 # mc0 unpermuted
                  wt[:, 0:128][:, perm],        # mc0 pair-permuted
                  wt[:, 128:256][:, perm]]      # mc1 pair-permuted
        for c_i, ch in enumerate(chunks):
            for kc in range(2):
                wqk[:, QK_SLOT[(w_i, c_i, kc)], :] = ch[kc * 128:(kc + 1) * 128]

    wvo = np.empty((128, 4, HID), dtype=np.float16)
    wv_s = wv_t * np.float32(R0)
    wo_s = wo_t
    wvo[:, 0:2] = wv_s.reshape(2, 128, HID).transpose(1, 0, 2)
    wvo[:, 2:4] = wo_s.reshape(2, 128, HID).transpose(1, 0, 2)

    in_maps = []
    for c in range(NCORES):
        in_maps.append({
            "x": np.ascontiguousarray(x[c * NB:(c + 1) * NB]),
            "y": np.ascontiguousarray(y[c * NB:(c + 1) * NB]),
            "wqk": wqk, "wvo": wvo,
            "b_out": bo,
        })

    res = run_bass_kernel_spmd(nc, in_maps, list(range(NCORES)))
    full = np.concatenate([res.results[i]["out"] for i in range(NCORES)], axis=0)
    return full.astype(np.float32).reshape(16, C, 32, 32)


# revision 33
# speedup vs baseline: 1.1797x; 1.0098x over previous
"""Cross-attention kernel for 8 trn2 NeuronCores.

Reference computation (per batch b of 16):
  q = Wq @ x, k = Wk @ y, v = Wv @ y          (1x1 convs as channel matmuls)
  q,k l2-normalized over the SPATIAL axis (per (h,d) row)
  sim = 10 * q^T k per head; attn = softmax_j(sim); o = attn @ v^T
  out = Wo @ o + b

Sharding: data-parallel over batch, 2 batches per core, weights replicated.

v4 design (ACT/PE/DVE co-roofline, built against the TimelineSim model):
  - S_T (q^T k) in fp8e4m3 + DoubleRow for heads 2..7: q raw (N(0,1) fits
    e4m3), k carries the combined l2 scale sq*sk*1024.  The DR pair layout
    [32p, 2pair, n] is produced WITHOUT a DRAM round-trip: the host permutes
    the Wq/Wk output columns so the projection PSUM partitions come out as
    [pair, ha, dlow]; two partition-base-offset copies then write the
    [64, 2, n] pair tile directly.
  - Heads 0-1 (first batch, mc0) use f16 S_T with unpermuted weights so the
    first exp starts ~8.5us (no quantize on the critical path); their qn
    copies run on the (otherwise idle) ACT engine.
  - Softmax normalize in ONE DVE op: the PV ones-block and Wv are pre-scaled
    by R0~=1/1027, so den*R0 ~= 1+-0.01 and one Newton step from the
    constant seed is exact to ~1e-4:  o = (den*R0 - 2) * (R0*EV) = -o_true,
    with the sign folded into Wo on the host.
  - All projection/quantize/v/zproj work is drip-fed into the 64 exp slots
    via per-jc fillers so the in-order PE queue never delays the next S_T.
  - Tail: zproj pieces run immediately per (mc, ih); the nb=1 bias-adds ride
    the ACT engine (Identity+bias, same act table as Exp); output DMA is f16.
  - PSUM: st [128,1024]f32 x2 (4 banks) + op [128,1024] x1 (2) + acc
    [128,512] x2 (2).
"""

import sys
from collections import deque

import numpy as np

if "/opt/trn_rl_repo" not in sys.path:
    sys.path.insert(0, "/opt/trn_rl_repo")

NB = 2        # batches per core
C = 256       # channels
N = 1024      # spatial (32*32)
HEADS = 4
DH = 64
HID = 256
NCORES = 8
MAGIC = 0x5F3759DF  # Quake fast inverse-sqrt seed
KGAIN = 1024.0      # power-of-two gain folded into k8; exp scale = 10/KGAIN
R0 = 1.0 / 1027.0   # Newton seed for 1/den (den ~= 1024 * (1 + E[s^2]/2))

_CACHE = {}
PHASES = []


def _mark(nc, label):
    PHASES.append((int(nc.get_next_instruction_name()[2:]), label))


def _quake_rsqrt(eng, pool, p_ap, out_ap, final_scale, iters=2):
    """out = rsqrt(p) * final_scale for [128,1] fp32 APs on engine `eng`.

    Quake seed + Newton iterations (1 iter: rel err ~2e-3; 2: ~1e-6).
    """
    from concourse import mybir

    i32 = mybir.dt.int32
    alu = mybir.AluOpType
    t = pool.tile([128, 1], mybir.dt.float32, tag="qk_rs_t", bufs=4)
    r = pool.tile([128, 1], mybir.dt.float32, tag="qk_rs_r", bufs=4)
    a = pool.tile([128, 1], mybir.dt.float32, tag="qk_rs_a", bufs=4)
    eng.tensor_scalar(t.bitcast(i32), p_ap.bitcast(i32), 1, None,
                      alu.logical_shift_right)
    eng.tensor_scalar(r.bitcast(i32), t.bitcast(i32), -1, MAGIC,
                      alu.mult, alu.add)
    cur = r
    if iters == 2:
        eng.scalar_tensor_tensor(a[:], r[:], r[:, 0:1], p_ap,
                                 alu.mult, alu.mult)
        eng.tensor_scalar(a[:], a[:], -0.5, 1.5, alu.mult, alu.add)
        eng.tensor_scalar(t[:], a[:], r[:, 0:1], None, alu.mult)
        cur = t
    eng.scalar_tensor_tensor(a[:], cur[:], cur[:, 0:1], p_ap,
                             alu.mult, alu.mult)
    eng.tensor_scalar(a[:], a[:], -0.5, 1.5, alu.mult, alu.add)
    eng.tensor_scalar(out_ap, a[:], cur[:, 0:1], final_scale,
                      alu.mult, alu.mult)


def _build_nc():
    from contextlib import ExitStack

    import concourse.tile as tile
    from concourse import bacc, mybir

    f32 = mybir.dt.float32
    f16 = mybir.dt.float16
    f8 = mybir.dt.float8e4
    alu = mybir.AluOpType
    EXP = mybir.ActivationFunctionType.Exp
    COPY = mybir.ActivationFunctionType.Copy
    IDENT = mybir.ActivationFunctionType.Identity
    DR = mybir.MatmulPerfMode.DoubleRow

    nc = bacc.Bacc("TRN2", target_bir_lowering=False)

    xin = nc.dram_tensor("x", [NB, C, N], f16, kind="ExternalInput")
    yin = nc.dram_tensor("y", [NB, C, N], f16, kind="ExternalInput")
    # wqk slots (s_qk below): [0:4] = startup mc0-unperm q/k x kc (loaded
    # first, small DMA); [4:8] = q mc0-perm/mc1-perm; [8:12] = k perm.
    wqk = nc.dram_tensor("wqk", [128, 12, 128], f16, kind="ExternalInput")
    # wvo slots: [kc0 wv, kc1 wv, kc0 wo, kc1 wo]; wv scaled by R0, wo by -1.
    wvo = nc.dram_tensor("wvo", [128, 4, HID], f16, kind="ExternalInput")
    bo = nc.dram_tensor("b_out", [2, 128, 1], f32, kind="ExternalInput")
    out = nc.dram_tensor("out", [NB, C, N], f16, kind="ExternalOutput")

    with tile.TileContext(nc) as tc, ExitStack() as ctx:
        consts = ctx.enter_context(tc.tile_pool(name="consts", bufs=1))
        big = ctx.enter_context(tc.tile_pool(name="big", bufs=2))
        sm = ctx.enter_context(tc.tile_pool(name="sm", bufs=4))
        ps = ctx.enter_context(tc.tile_pool(name="ps", bufs=2, space="PSUM"))

        # ---- constants + input DMA ------------------------------------
        wqk_sb = consts.tile([128, 12, 128], f16, tag="wqk")
        wvo_sb = consts.tile([128, 4, HID], f16, tag="wvo")
        b_sb = consts.tile([128, 2, 1], f32, tag="bo")
        xts, yts = [], []
        for nb in range(NB):
            xts.append(big.tile([128, 2, N], f16, tag="xt", bufs=2,
                                name=f"xt{nb}"))
            yts.append(big.tile([128, 2, N], f16, tag="yt", bufs=2,
                                name=f"yt{nb}"))
        # warm the ACT exp table while input DMAs are in flight
        warm = sm.tile([128, 1], f32, tag="warm", bufs=1)
        nc.vector.memset(warm[:], 0.0)
        nc.scalar.activation(out=warm[:], in_=warm[:], func=EXP, scale=1.0)
        # startup-critical loads first (k-side before q-side), column-split
        # so the first projection matmuls start one DMA earlier.
        nc.sync.dma_start(out=wqk_sb[:, 0:4, :], in_=wqk[:, 0:4, :])
        yr0 = yin[0].rearrange("(kc p) n -> p kc n", p=128)
        xr0 = xin[0].rearrange("(kc p) n -> p kc n", p=128)
        nc.sync.dma_start(out=yts[0][:, :, 0:512], in_=yr0[:, :, 0:512])
        nc.sync.dma_start(out=yts[0][:, :, 512:1024], in_=yr0[:, :, 512:1024])
        nc.sync.dma_start(out=xts[0][:, :, 0:512], in_=xr0[:, :, 0:512])
        nc.sync.dma_start(out=xts[0][:, :, 512:1024], in_=xr0[:, :, 512:1024])
        nc.sync.dma_start(out=wvo_sb[:], in_=wvo[:])
        nc.sync.dma_start(out=wqk_sb[:, 4:12, :], in_=wqk[:, 4:12, :])
        nc.sync.dma_start(out=b_sb[:], in_=bo.rearrange("kc p n -> p kc n"))

        # ---- PE p-state warmup (rotating acc tiles; runs in DMA wait) -
        wsrc = big.tile([128, 512], f16, tag="wsrc", bufs=1, name="wsrc")
        nc.gpsimd.memset(wsrc[:], 0.0)
        for _ in range(7):
            wp = ps.tile([128, 512], f32, tag="acc", bufs=2, name="wp")
            nc.tensor.matmul(wp[:], wsrc[:, 0:128], wsrc[:],
                             start=True, stop=True)

        # ---- persistent attention tiles -------------------------------
        qn = big.tile([128, N], f16, tag="qn", bufs=1, name="qn")
        kn = big.tile([128, N], f16, tag="kn", bufs=1, name="kn")
        q8s, k8s = {}, {}
        for key in ((0, 1), (1, 0), (1, 1)):
            q8s[key] = big.tile([64, 2, N], f8, tag="q8", bufs=3,
                                name=f"q8_{key[0]}{key[1]}")
            k8s[key] = big.tile([64, 2, N], f8, tag="k8", bufs=3,
                                name=f"k8_{key[0]}{key[1]}")
        o0 = big.tile([128, 2, N], f16, tag="osb", bufs=2, name="o0")
        o1 = big.tile([128, 2, N], f16, tag="osb", bufs=2, name="o1")
        vts0 = [big.tile([128, 4, 128], f16, tag="vt", bufs=16,
                         name=f"vt0_{jc}") for jc in range(8)]
        vts1 = [big.tile([128, 4, 128], f16, tag="vt", bufs=16,
                         name=f"vt1_{jc}") for jc in range(8)]
        # ones-blocks are constant and disjoint from the v region: set all
        # 16 upfront while Pool is idle (frees Pool for mid-stream work)
        for vt in vts0 + vts1:
            nc.gpsimd.memset(vt[:, :, 64:128], R0)

        QK_SLOT = {(1, 0, 0): 0, (1, 0, 1): 1, (0, 0, 0): 2, (0, 0, 1): 3,
                   (0, 1, 0): 4, (0, 1, 1): 5, (0, 2, 0): 6, (0, 2, 1): 7,
                   (1, 1, 0): 8, (1, 1, 1): 9, (1, 2, 0): 10, (1, 2, 1): 11}

        def s_qk(w, chunk, kc):
            return QK_SLOT[(w, chunk, kc)]

        # ---- (0,0) f16 startup chain ----------------------------------
        # qp lives in the two acc halves (its readers -- stats + the ACT qn
        # copies -- are off the k-side critical chain); kp lives in the
        # until-ST0-idle st pool so it never waits on the qn copies.
        _mark(nc, "startup chain")
        # k projection first (y loads first); kp in the until-ST0-idle st
        # pool so it never waits on the qn ACT copies.
        kp = ps.tile([128, N], f32, tag="st", bufs=2, name="kp00")
        for ih in range(2):
            for kc in range(2):
                nc.tensor.matmul(kp[:, ih * 512:(ih + 1) * 512],
                                 wqk_sb[:, s_qk(1, 0, kc), :],
                                 yts[0][:, kc, ih * 512:(ih + 1) * 512],
                                 start=(kc == 0), stop=(kc == 1))
        stk = sm.tile([128, 2, 6], f32, tag="stk", bufs=4, name="stk00")
        nc.vector.bn_stats(out=stk[:, 0, :], in_=kp[:, 0:512])
        qph = []
        for ih in range(2):
            qp = ps.tile([128, 512], f32, tag="acc", bufs=2, name=f"qp00_{ih}")
            for kc in range(2):
                nc.tensor.matmul(qp[:], wqk_sb[:, s_qk(0, 0, kc), :],
                                 xts[0][:, kc, ih * 512:(ih + 1) * 512],
                                 start=(kc == 0), stop=(kc == 1))
            qph.append(qp)
        nc.vector.bn_stats(out=stk[:, 1, :], in_=kp[:, 512:1024])
        stq = sm.tile([128, 2, 6], f32, tag="stq", bufs=4, name="stq00")
        for ih in range(2):
            nc.vector.bn_stats(out=stq[:, ih, :], in_=qph[ih][:])
        # qn head-0 copies on ACT (idle until first exp); head-1 copies on
        # DVE (needed only by head 1, ~8us later) so they never preempt exps
        for ih in range(2):
            nc.scalar.activation(
                out=qn[0:64, ih * 512:(ih + 1) * 512],
                in_=qph[ih][0:64, :], func=COPY, scale=1.0)
        with tc.tile_wait_until(0.012):
            for ih in range(2):
                nc.vector.tensor_copy(
                    qn[64:128, ih * 512:(ih + 1) * 512],
                    qph[ih][64:128, :])
        mvq = sm.tile([128, 2], f32, tag="mvq", bufs=4, name="mvq00")
        mvk = sm.tile([128, 2], f32, tag="mvk", bufs=4, name="mvk00")
        nc.vector.bn_aggr(out=mvq[:], in_=stq[:])
        nc.vector.bn_aggr(out=mvk[:], in_=stk[:])
        uq = sm.tile([128, 1], f32, tag="uq", bufs=4, name="uq00")
        pqk = sm.tile([128, 1], f32, tag="pqk", bufs=4, name="pqk00")
        nc.vector.scalar_tensor_tensor(uq[:], mvq[:, 0:1], mvq[:, 0:1],
                                       mvq[:, 1:2], alu.mult, alu.add)
        nc.vector.scalar_tensor_tensor(pqk[:], mvk[:, 0:1], mvk[:, 0:1],
                                       mvk[:, 1:2], alu.mult, alu.add)
        nc.vector.tensor_tensor(pqk[:], pqk[:], uq[:], alu.mult)
        sck16 = sm.tile([128, 1], f32, tag="sck", bufs=4, name="sck00")
        _quake_rsqrt(nc.vector, sm, pqk[:], sck16[:], 1.0 / float(N),
                     iters=1)
        # kn scale: jc0 block first so the first S_T can fire, then the rest
        nc.vector.tensor_scalar(kn[:, 0:128], kp[:, 0:128],
                                sck16[:, 0:1], None, alu.mult)

        # f16 S_T for heads 0-1
        def st_mm_f16(ha):
            def mm(jc, quad=False):
                if quad:
                    sth = []
                    for ih in range(2):
                        st = ps.tile([128, 512], f32, tag="acc", bufs=2,
                                     name="bst")
                        nc.tensor.matmul(
                            st[:],
                            kn[64 * ha:64 * (ha + 1), jc * 128:(jc + 1) * 128],
                            qn[64 * ha:64 * (ha + 1), ih * 512:(ih + 1) * 512],
                            start=True, stop=True)
                        sth.append(st)
                    return sth
                st = ps.tile([128, N], f32, tag="st", bufs=2)
                for ih in range(2):
                    nc.tensor.matmul(
                        st[:, ih * 512:(ih + 1) * 512],
                        kn[64 * ha:64 * (ha + 1), jc * 128:(jc + 1) * 128],
                        qn[64 * ha:64 * (ha + 1), ih * 512:(ih + 1) * 512],
                        start=True, stop=True)
                return st
            return mm

        st00 = st_mm_f16(0)
        first_st_00 = st00(0)
        nc.vector.tensor_scalar(kn[:, 128:512], kp[:, 128:512],
                                sck16[:, 0:1], None, alu.mult)
        nc.vector.tensor_scalar(kn[:, 512:1024], kp[:, 512:1024],
                                sck16[:, 0:1], None, alu.mult)

        # ---- startup v-projection (jc 0..3) through the op-pool tile --
        vpb = ps.tile([128, N], f32, tag="op", bufs=1, name="vpb")
        for jc in range(4):
            for kc in range(2):
                nc.tensor.matmul(vpb[:, jc * 256:(jc + 1) * 256],
                                 yts[0][:, kc, jc * 128:(jc + 1) * 128],
                                 wvo_sb[:, kc, :],
                                 start=(kc == 0), stop=(kc == 1))
        # gate the vt copies past the startup DVE chain (~9.5us) so the
        # readiness-greedy scheduler can't interleave them into it
        with tc.tile_wait_until(0.0095):
            for jc in range(4):
                nc.vector.tensor_copy(
                    vts0[jc][:, :, 0:64],
                    vpb[:, jc * 256:(jc + 1) * 256].rearrange("p (h d) -> p h d", h=4))

        # ---- filler pieces --------------------------------------------
        def load_b1():
            nc.sync.dma_start(out=yts[1][:], in_=yin[1].rearrange("(kc p) n -> p kc n", p=128))
            nc.sync.dma_start(out=xts[1][:], in_=xin[1].rearrange("(kc p) n -> p kc n", p=128))

        def proj_v_piece(nb, jc, vts_nb):
            vp = ps.tile([128, 512], f32, tag="acc", bufs=2)
            for kc in range(2):
                nc.tensor.matmul(vp[:, 0:HID],
                                 yts[nb][:, kc, jc * 128:(jc + 1) * 128],
                                 wvo_sb[:, kc, :],
                                 start=(kc == 0), stop=(kc == 1))
            nc.vector.tensor_copy(
                vts_nb[jc][:, :, 0:64],
                vp[:, 0:HID].rearrange("p (h d) -> p h d", h=4))

        def fp8_proj_pieces(nb, mc, q8t, k8t):
            """10 filler closures: project q/k (permuted cols), l2 stats,
            quantize into DR pair tiles."""
            st_ = {}
            tag = f"{nb}{mc}"
            chunk = 1 if mc == 0 else 2

            def qmm(ih):
                def f():
                    qp = ps.tile([128, 512], f32, tag="acc", bufs=2)
                    for kc in range(2):
                        nc.tensor.matmul(qp[:], wqk_sb[:, s_qk(0, chunk, kc), :],
                                         xts[nb][:, kc, ih * 512:(ih + 1) * 512],
                                         start=(kc == 0), stop=(kc == 1))
                    st_[f"qp{ih}"] = qp
                    if ih == 0:
                        st_["stq"] = sm.tile([128, 2, 6], f32, tag="stq",
                                             bufs=4, name=f"stq{tag}")
                    else:
                        nc.vector.bn_stats(out=st_["stq"][:, 0, :],
                                           in_=st_["qp0"][:])
                return f

            def qfin(ih):
                def f():
                    qp = st_[f"qp{ih}"]
                    if ih == 1:
                        nc.vector.bn_stats(out=st_["stq"][:, 1, :], in_=qp[:])
                    for pair in range(2):
                        nc.vector.tensor_copy(
                            q8t[:, pair, ih * 512:(ih + 1) * 512],
                            qp[64 * pair:64 * (pair + 1), :])
                return f

            def kmm(ih):
                def f():
                    kp = ps.tile([128, 512], f32, tag="acc", bufs=2)
                    for kc in range(2):
                        nc.tensor.matmul(kp[:], wqk_sb[:, s_qk(1, chunk, kc), :],
                                         yts[nb][:, kc, ih * 512:(ih + 1) * 512],
                                         start=(kc == 0), stop=(kc == 1))
                    st_[f"kp{ih}"] = kp
                    if ih == 0:
                        st_["stk"] = sm.tile([128, 2, 6], f32, tag="stk",
                                             bufs=4, name=f"stk{tag}")
                    else:
                        nc.vector.bn_stats(out=st_["stk"][:, 0, :],
                                           in_=st_["kp0"][:])
                return f

            def kcomb():
                nc.vector.bn_stats(out=st_["stk"][:, 1, :], in_=st_["kp1"][:])
                mvq_ = sm.tile([128, 2], f32, tag="mvq", bufs=4)
                mvk_ = sm.tile([128, 2], f32, tag="mvk", bufs=4)
                nc.vector.bn_aggr(out=mvq_[:], in_=st_["stq"][:])
                nc.vector.bn_aggr(out=mvk_[:], in_=st_["stk"][:])
                uq_ = sm.tile([128, 1], f32, tag="uq", bufs=4)
                pqk_ = sm.tile([128, 1], f32, tag="pqk", bufs=4)
                nc.vector.scalar_tensor_tensor(uq_[:], mvq_[:, 0:1], mvq_[:, 0:1],
                                               mvq_[:, 1:2], alu.mult, alu.add)
                nc.vector.scalar_tensor_tensor(pqk_[:], mvk_[:, 0:1], mvk_[:, 0:1],
                                               mvk_[:, 1:2], alu.mult, alu.add)
                nc.vector.tensor_tensor(pqk_[:], pqk_[:], uq_[:], alu.mult)
                sck = sm.tile([128, 1], f32, tag="sck", bufs=4, name=f"sck{tag}")
                # k8 = kp * sck * KGAIN; KGAIN/N == 1.0
                _quake_rsqrt(nc.vector, sm, pqk_[:], sck[:], KGAIN / float(N),
                             iters=1)
                st_["sck"] = sck

            def kfin(ih):
                def f():
                    kp = st_[f"kp{ih}"]
                    sck = st_["sck"]
                    for pair in range(2):
                        nc.vector.tensor_scalar(
                            k8t[:, pair, ih * 512:(ih + 1) * 512],
                            kp[64 * pair:64 * (pair + 1), :],
                            sck[64 * pair:64 * (pair + 1), 0:1], None, alu.mult)
                return f

            return [qmm(0), qmm(1), qfin(0), qfin(1),
                    kmm(0), kmm(1), kcomb, kfin(0), kfin(1)]

        def zproj_piece(nb, o_sb, mc, ih, on_act):
            def f():
                sl = slice(ih * 512, (ih + 1) * 512)
                msl = slice(mc * 128, (mc + 1) * 128)
                zp = ps.tile([128, 512], f32, tag="acc", bufs=2)
                nc.tensor.matmul(zp[:], wvo_sb[:, 2, msl], o_sb[:, 0, sl],
                                 start=True, stop=False)
                nc.tensor.matmul(zp[:], wvo_sb[:, 3, msl], o_sb[:, 1, sl],
                                 start=False, stop=True)
                zs = big.tile([128, 512], f16, tag="zs", bufs=4)
                if on_act:
                    nc.scalar.activation(out=zs[:], in_=zp[:], func=IDENT,
                                         bias=b_sb[:, mc, 0:1], scale=1.0)
                else:
                    nc.vector.tensor_scalar(zs[:], zp[:], b_sb[:, mc, 0:1],
                                            None, alu.add)
                nc.sync.dma_start(out=out[nb, msl, sl], in_=zs[:])
            return f

        # ---- heads ----------------------------------------------------
        def st_mm_fp8(nb, mc, ha):
            q8t, k8t = q8s[(nb, mc)], k8s[(nb, mc)]

            def mm(jc, quad=False):
                if quad:
                    # quad tiles park in acc halves: the st pool's 2-deep
                    # exp pipeline stays unshifted (its consumer is DVE)
                    sth = []
                    for ih in range(2):
                        st = ps.tile([128, 512], f32, tag="acc", bufs=2,
                                     name="qst")
                        nc.tensor.matmul(
                            st[:],
                            k8t[ha * 32:(ha + 1) * 32, :, jc * 128:(jc + 1) * 128],
                            q8t[ha * 32:(ha + 1) * 32, :, ih * 512:(ih + 1) * 512],
                            start=True, stop=True, perf_mode=DR)
                        sth.append(st)
                    return sth
                st = ps.tile([128, N], f32, tag="st", bufs=2)
                for ih in range(2):
                    nc.tensor.matmul(
                        st[:, ih * 512:(ih + 1) * 512],
                        k8t[ha * 32:(ha + 1) * 32, :, jc * 128:(jc + 1) * 128],
                        q8t[ha * 32:(ha + 1) * 32, :, ih * 512:(ih + 1) * 512],
                        start=True, stop=True, perf_mode=DR)
                return st
            return mm

        made = []

        def make_head(nb, h, vts_nb, o_sb, f16_path=False):
            mc, ha = h // 2, h % 2
            if f16_path:
                st_mm = st_mm_f16(ha)
                scale = 10.0
            else:
                st_mm = st_mm_fp8(nb, mc, ha)
                scale = 10.0 / KGAIN
            hc = {"nb": nb, "h": h, "mc": mc, "ha": ha, "vts": vts_nb,
                  "o_sb": o_sb, "st_mm": st_mm, "first_st": None,
                  "scale": scale, "idx": len(made)}
            made.append(hc)
            return hc

        fillers = deque()

        def run_head(hc, next_hc):
            h = hc["h"]
            _mark(nc, f"head(b{hc['nb']},h{h}) start")
            op = ps.tile([128, N], f32, tag="op", bufs=1, name="op")
            sts = [hc["first_st"] if hc["first_st"] is not None
                   else hc["st_mm"](0)]
            deferred_pv = []
            quad_jcs = {q[1] for q in QUAD if q[0] == hc["idx"]}
            last_jc = 7 if 7 not in quad_jcs else max(
                j for j in range(8) if j not in quad_jcs)
            for jc in range(8):
                et = big.tile([128, N], f16, tag="et", bufs=8, name="et")
                if hc.get("tail") and jc == 7:
                    # split the last exp per-ih so the tail PV/norm/zproj
                    # pipeline starts half an exp earlier
                    for ih in range(2):
                        sl = slice(ih * 512, (ih + 1) * 512)
                        nc.scalar.activation(out=et[:, sl], in_=sts[jc][:, sl],
                                             func=EXP, scale=hc["scale"])
                elif (hc["idx"], jc) in QUAD:
                    # quadratic exp offload: e ~= 1 + u + u^2/2, u = scale*st
                    # s16 on DVE (PSUM read), the polynomial on Pool f16
                    a = hc["scale"]
                    s16 = big.tile([128, N], f16, tag="s16", bufs=2,
                                   name="s16")
                    t2 = big.tile([128, N], f16, tag="t2", bufs=2, name="t2")
                    with tc.high_priority():
                        for ih in range(2):
                            sl = slice(ih * 512, (ih + 1) * 512)
                            nc.vector.tensor_copy(s16[:, sl],
                                                  sts[jc][ih][:])
                    nc.gpsimd.tensor_scalar(t2[:], s16[:], 2.0 / a, None,
                                            alu.add)
                    nc.gpsimd.tensor_tensor(t2[:], t2[:], s16[:], alu.mult)
                    nc.gpsimd.tensor_scalar(et[:], t2[:], a * a / 2.0, 1.0,
                                            alu.mult, alu.add)
                elif isinstance(sts[jc], list):
                    for ih in range(2):
                        sl = slice(ih * 512, (ih + 1) * 512)
                        nc.scalar.activation(out=et[:, sl],
                                             in_=sts[jc][ih][:],
                                             func=EXP, scale=hc["scale"])
                else:
                    nc.scalar.activation(out=et[:], in_=sts[jc][:], func=EXP,
                                         scale=hc["scale"])
                if jc < 7:
                    sts.append(hc["st_mm"](jc + 1,
                                           quad=(jc + 1) in quad_jcs))
                if jc == 6 and next_hc is not None:
                    next_hc["first_st"] = next_hc["st_mm"](0)
                if fillers and jc >= 1:
                    _mark(nc, f"head(b{hc['nb']},h{h}) filler jc{jc}")
                    fillers.popleft()()
                vt = hc["vts"][jc]
                if jc in quad_jcs:
                    # quad et arrives ~5us late (Pool chain); emit its PV at
                    # the END of the accumulation group (order is free) so it
                    # never blocks the other PVs
                    def pv(jc=jc, et=et, vt=vt):
                        for ih in range(2):
                            nc.tensor.matmul(
                                op[:, ih * 512:(ih + 1) * 512],
                                vt[:, h, :], et[:, ih * 512:(ih + 1) * 512],
                                start=False, stop=True)
                    deferred_pv.append(pv)
                    continue
                for ih in range(2):
                    nc.tensor.matmul(
                        op[:, ih * 512:(ih + 1) * 512],
                        vt[:, h, :],
                        et[:, ih * 512:(ih + 1) * 512],
                        start=(jc == 0), stop=(jc == last_jc and
                                               not deferred_pv))
            for pv in deferred_pv:
                pv()
            # Newton normalize (hw allows only one PSUM read per op):
            # one [128,1024] f16 copy lands EV + den rows in SBUF (DVE cost
            # is partition-blind), Pool computes 2-den*R0, DVE multiplies at
            # f16 2x: o = (2 - den*R0) * (R0*EV) ~= o_true.
            hr = 64 * hc["ha"]
            ihs = ((0, 1),) if not hc.get("tail") else ((0,), (1,))
            for grp in ihs:
                sl = slice(grp[0] * 512, (grp[-1] + 1) * 512)
                w = len(grp) * 512
                ocp = big.tile([128, N], f16, tag="ocp", bufs=2, name="ocp")
                d2 = big.tile([64, N], f16, tag="d2", bufs=2, name="d2")
                with tc.high_priority():
                    nc.vector.tensor_copy(ocp[:, sl], op[:, sl])
                # polynomial on Pool only for batch-0 heads (o0 gates only
                # the loose zproj fillers); batch-1 norms gate the tail's
                # zproj and stay on DVE
                eng = nc.vector if hc["nb"] == 1 else nc.gpsimd
                eng.tensor_scalar(d2[:, sl], ocp[64:128, sl],
                                  2.0, -1.0, alu.subtract, alu.mult)
                eng.tensor_tensor(
                    hc["o_sb"][hr:hr + 64, hc["mc"], sl],
                    ocp[0:64, sl], d2[:, sl], alu.mult)
            _mark(nc, f"head(b{hc['nb']},h{h}) end")

        # quadratic-exp offload slots (head idx, jc): back half, where
        # DVE/Pool have slack
        QUAD = set()
        hcs = [make_head(0, 0, vts0, o0, f16_path=True),
               make_head(0, 1, vts0, o0, f16_path=True),
               make_head(0, 2, vts0, o0),
               make_head(0, 3, vts0, o0),
               make_head(1, 0, vts1, o1),
               make_head(1, 1, vts1, o1),
               make_head(1, 2, vts1, o1),
               make_head(1, 3, vts1, o1)]
        hcs[7]["tail"] = True
        hcs[0]["first_st"] = first_st_00

        # filler order tuned against per-head deadlines: each fp8 proj's
        # quantize must land before its head pair starts; v(nb) tiles before
        # that batch's PVs reach them.
        p01 = fp8_proj_pieces(0, 1, q8s[(0, 1)], k8s[(0, 1)])
        p10 = fp8_proj_pieces(1, 0, q8s[(1, 0)], k8s[(1, 0)])
        p11 = fp8_proj_pieces(1, 1, q8s[(1, 1)], k8s[(1, 1)])
        vp0 = [lambda jc=jc: proj_v_piece(0, jc, vts0) for jc in range(4, 8)]
        vp1 = [lambda jc=jc: proj_v_piece(1, jc, vts1) for jc in range(8)]
        z0 = [zproj_piece(0, o0, mc, ih, on_act=False)
              for mc in range(2) for ih in range(2)]
        fillers.append(load_b1)
        fillers.extend([p01[0], vp0[0], p01[1], vp0[1], p01[2], vp0[2],
                        p01[3], vp0[3]])
        fillers.extend(p01[4:9])
        fillers.extend(p10)
        fillers.extend([p11[0], vp1[0], vp1[1], p11[1], vp1[2], vp1[3],
                        p11[2], vp1[4], vp1[5], p11[3], vp1[6], vp1[7]])
        fillers.extend(p11[4:9])
        fillers.extend(z0)

        for i in range(8):
            run_head(hcs[i], hcs[i + 1] if i < 7 else None)
        # drain leftover fillers (shouldn't happen), then tail zproj(1):
        # per-ih shared staging tile, bias-add split DVE/ACT, one DMA per ih
        while fillers:
            fillers.popleft()()
        for ih in range(2):
            sl = slice(ih * 512, (ih + 1) * 512)
            zs2 = big.tile([128, 2, 512], f16, tag="zs2", bufs=2, name="zs2")
            for mc in range(2):
                msl = slice(mc * 128, (mc + 1) * 128)
                zp = ps.tile([128, 512], f32, tag="acc", bufs=2, name="zp")
                nc.tensor.matmul(zp[:], wvo_sb[:, 2, msl], o1[:, 0, sl],
                                 start=True, stop=False)
                nc.tensor.matmul(zp[:], wvo_sb[:, 3, msl], o1[:, 1, sl],
                                 start=False, stop=True)
                if mc == 1:
                    nc.scalar.activation(out=zs2[:, mc, :], in_=zp[:],
                                         func=IDENT, bias=b_sb[:, mc, 0:1],
                                         scale=1.0)
                else:
                    nc.vector.tensor_scalar(zs2[:, mc, :], zp[:],
                                            b_sb[:, mc, 0:1], None, alu.add)
            nc.sync.dma_start(
                out=out[1].rearrange("(kc p) n -> p kc n", p=128)[:, :, sl],
                in_=zs2[:])

    nc.finalize()
    return nc


def _get_nc():
    if "nc" not in _CACHE:
        _CACHE["nc"] = _build_nc()
    return _CACHE["nc"]


def _perm128():
    """Permutation for DR pair layout: PSUM partition p holds output channel
    perm(p) = ha*64 + dlow + 32*pair with ha=(p%64)//32, dlow=p%32, pair=p//64."""
    p = np.arange(128)
    return ((p % 64) // 32) * 64 + (p % 32) + 32 * (p // 64)


def kernel(x, y, w_qkv, w_out, b_out):
    from concourse.bass_utils import run_bass_kernel_spmd

    nc = _get_nc()

    x = np.asarray(x, dtype=np.float32).reshape(16, C, N).astype(np.float16)
    y = np.asarray(y, dtype=np.float32).reshape(16, C, N).astype(np.float16)
    w_qkv = np.asarray(w_qkv, dtype=np.float32)
    wq_t = np.ascontiguousarray(w_qkv[0:HID].T)          # [256 c, 256 out]
    wk_t = np.ascontiguousarray(w_qkv[HID:2 * HID].T)
    wv_t = np.ascontiguousarray(w_qkv[2 * HID:3 * HID].T)
    wo_t = np.ascontiguousarray(np.asarray(w_out, dtype=np.float32).T)
    bo = np.ascontiguousarray(
        np.asarray(b_out, dtype=np.float32).reshape(2, 128, 1))

    QK_SLOT = {(1, 0, 0): 0, (1, 0, 1): 1, (0, 0, 0): 2, (0, 0, 1): 3,
               (0, 1, 0): 4, (0, 1, 1): 5, (0, 2, 0): 6, (0, 2, 1): 7,
               (1, 1, 0): 8, (1, 1, 1): 9, (1, 2, 0): 10, (1, 2, 1): 11}
    perm = _perm128()
    wqk = np.empty((128, 12, 128), dtype=np.float16)
    for w_i, wt in ((0, wq_t), (1, wk_t)):
        chunks = [wt[:, 0:128],                 # mc0 unpermuted
                  wt[:, 0:128][:, perm],        # mc0 pair-permuted
                  wt[:, 128:256][:, perm]]      # mc1 pair-permuted
        for c_i, ch in enumerate(chunks):
            for kc in range(2):
                wqk[:, QK_SLOT[(w_i, c_i, kc)], :] = ch[kc * 128:(kc + 1) * 128]

    wvo = np.empty((128, 4, HID), dtype=np.float16)
    wv_s = wv_t * np.float32(R0)
    wo_s = wo_t
    wvo[:, 0:2] = wv_s.reshape(2, 128, HID).transpose(1, 0, 2)
    wvo[:, 2:4] = wo_s.reshape(2, 128, HID).transpose(1, 0, 2)

    in_maps = []
    for c in range(NCORES):
        in_maps.append({
            "x": np.ascontiguousarray(x[c * NB:(c + 1) * NB]),
            "y": np.ascontiguousarray(y[c * NB:(c + 1) * NB]),
            "wqk": wqk, "wvo": wvo,
            "b_out": bo,
        })

    res = run_bass_kernel_spmd(nc, in_maps, list(range(NCORES)))
    full = np.concatenate([res.results[i]["out"] for i in range(NCORES)], axis=0)
    return full.astype(np.float32).reshape(16, C, 32, 32)
